# revision 60
# baseline (speedup 1.0000x reference)
"""Distributed causal-attention kernel for 8 TRN2 NeuronCores.

Reference computation (B=2, S=2048, E=1024, H=16, D=64):
  qkv = x @ w_qkv; split; rope(q), rope(k); causal softmax attention; out @ w_out

Sharding: data-parallel over batch (cores 0-3 -> batch 0, 4-7 -> batch 1),
tensor-parallel over heads (4 heads per core). Each core computes a partial
[S, E] out-projection; the host sums the 4 partials per batch.

All matmuls run in bf16 on the TensorEngine with fp32 PSUM accumulation.
Host-side prep pre-transposes x to [E, S], permutes q/k weight columns so the
RoPE pair interleave becomes two contiguous 32-row blocks per head, and
pre-tiles everything into [128, *] DMA-friendly layouts.
"""

import sys
import types

sys.path.insert(0, "/opt/trn_rl_repo")

import numpy as np
import ml_dtypes

BF16 = ml_dtypes.bfloat16

B, S, E, H, D = 2, 2048, 1024, 16, 64
N_CORES = 8
HPC = 4           # heads per core
DHC = HPC * D     # 256 head-dims per core
KT = E // 128     # 8 contraction tiles for the qkv projection
ST = S // 128     # 16 sequence tiles of 128
SC = S // 512     # 4 sequence chunks of 512
VW = D + 1        # 65: v plus the ones column (softmax denominator)


def _inject_axon_hooks():
    """Register the NTFF profile hook missing from this image's antenv so
    trace=True (or BASS_TRACE=1 in the env) doesn't crash run_bass_kernel_spmd."""
    if "antenv.axon_hooks" in sys.modules:
        return
    try:
        import antenv  # noqa: F401
    except Exception:
        return
    mod = types.ModuleType("antenv.axon_hooks")
    mod._hook = None

    def set_axon_ntff_profile_hook(h):
        mod._hook = h

    def get_axon_ntff_profile_hook():
        return mod._hook

    mod.set_axon_ntff_profile_hook = set_axon_ntff_profile_hook
    mod.get_axon_ntff_profile_hook = get_axon_ntff_profile_hook
    sys.modules["antenv.axon_hooks"] = mod
    try:
        from trn_agent_boot.trn_boot import _ntff_profile_via_ctypes

        mod._hook = _ntff_profile_via_ctypes("/opt/axon/libaxon_pjrt.so")
    except Exception:
        pass


def build():
    import concourse.bass as bass  # noqa: F401
    import concourse.mybir as mybir
    import concourse.tile as tile
    from concourse import bacc

    f32 = mybir.dt.float32
    bf16 = mybir.dt.bfloat16
    Exp = mybir.ActivationFunctionType.Exp

    nc = bacc.Bacc("TRN2", target_bir_lowering=False, debug=False,
                   num_devices=N_CORES)

    xt_e = nc.dram_tensor("xt", [128, KT * S], bf16, kind="ExternalInput").ap()
    wq_e = nc.dram_tensor("wq", [128, KT * DHC], bf16, kind="ExternalInput").ap()
    wk_e = nc.dram_tensor("wk", [128, KT * DHC], bf16, kind="ExternalInput").ap()
    wv_e = nc.dram_tensor("wv", [128, KT * DHC], bf16, kind="ExternalInput").ap()
    wo_e = nc.dram_tensor("wo", [128, 2 * E], bf16, kind="ExternalInput").ap()
    cr_e = nc.dram_tensor("crep", [128, S], f32, kind="ExternalInput").ap()
    sr_e = nc.dram_tensor("srep", [128, S], f32, kind="ExternalInput").ap()
    mk_e = nc.dram_tensor("mask", [128, 128], bf16, kind="ExternalInput").ap()
    out_e = nc.dram_tensor("out", [S, E], bf16, kind="ExternalOutput").ap()

    with tile.TileContext(nc) as tc:
        with (
            tc.tile_pool(name="static", bufs=1) as static,
            tc.tile_pool(name="tmp", bufs=2) as tmpp,
            tc.tile_pool(name="expp", bufs=32) as expp,
            tc.tile_pool(name="outp", bufs=4) as outp,
            tc.tile_pool(name="bcp", bufs=2) as bcp,
            tc.tile_pool(name="psacc", bufs=2, space="PSUM") as psacc,
            tc.tile_pool(name="psav", bufs=2, space="PSUM") as psav,
        ):
            xt = static.tile([128, KT * S], bf16, tag="xt")
            wq = static.tile([128, KT * DHC], bf16, tag="wq")
            wk = static.tile([128, KT * DHC], bf16, tag="wk")
            wv = static.tile([128, KT * DHC], bf16, tag="wv")
            wo = static.tile([128, 2 * E], bf16, tag="wo")
            crep = static.tile([128, S], f32, tag="crep")
            srep = static.tile([128, S], f32, tag="srep")
            mask = static.tile([128, 128], bf16, tag="mask")
            qs = [static.tile([128, S], bf16, tag=f"q{m}", name=f"q{m}") for m in range(2)]
            ks = [static.tile([128, S], bf16, tag=f"k{m}", name=f"k{m}") for m in range(2)]
            vsb = static.tile([128, ST * HPC, VW], bf16, tag="v")
            attn = [static.tile([128, S], bf16, tag=f"at{m}", name=f"at{m}") for m in range(2)]

            # Weights first (small, they gate the first matmuls), then xt
            # split into per-k-tile chunks so the k-accumulation can start
            # before the whole 4MB lands.
            nc.sync.dma_start(out=wq[:, :], in_=wq_e[:, :])
            nc.sync.dma_start(out=wk[:, :], in_=wk_e[:, :])
            nc.sync.dma_start(out=wv[:, :], in_=wv_e[:, :])
            nc.sync.dma_start(out=crep[:, :], in_=cr_e[:, :])
            nc.sync.dma_start(out=srep[:, :], in_=sr_e[:, :])
            nc.sync.dma_start(out=mask[:, :], in_=mk_e[:, :])
            # column-slice-major: all k-chunks of s-cols [512c5, 512c5+512)
            # land together, so the first qk tile (c5=0) completes ~14us
            # earlier than with whole-chunk streaming.
            for c5 in range(4):
                for k in range(KT):
                    nc.sync.dma_start(
                        out=xt[:, k * S + c5 * 512: k * S + (c5 + 1) * 512],
                        in_=xt_e[:, k * S + c5 * 512: k * S + (c5 + 1) * 512])
            nc.sync.dma_start(out=wo[:, :], in_=wo_e[:, :])

            nc.vector.memset(vsb[:, :, D:VW], 1.0)

            # ---- q, k projections + RoPE, [d, s] layout, 1024-wide chunks ----
            # m-tile rows: [hA_even(32) | hA_odd(32) | hB_even(32) | hB_odd(32)]
            def qk_tile(dst, w, mt, ch):
                # 512-wide halves on their own small psum tag so attention's
                # scores tiles never contend with rope-held qkv tiles.
                for hf in range(2):
                    c5 = 2 * ch + hf
                    ps = psacc.tile([128, 512], f32, tag="qp", name="qp")
                    for k in range(KT):
                        nc.tensor.matmul(
                            ps[:, :],
                            lhsT=w[:, k * DHC + mt * 128: k * DHC + (mt + 1) * 128],
                            rhs=xt[:, k * S + c5 * 512: k * S + (c5 + 1) * 512],
                            start=(k == 0), stop=(k == KT - 1),
                        )
                    # ta (SBUF) = q*cos, then ps (PSUM) *= sin in place;
                    # the rotate combines then always mix one SBUF and one
                    # PSUM operand, which the BIR verifier allows at
                    # different base partitions (SB+SB it does not).
                    ta = tmpp.tile([128, 512], f32, tag="ta")
                    c_s = crep[:, c5 * 512:(c5 + 1) * 512]
                    s_s = srep[:, c5 * 512:(c5 + 1) * 512]
                    nc.vector.tensor_mul(ta[:, :], ps[:, :], c_s)
                    nc.vector.tensor_mul(ps[:, :], ps[:, :], s_s)
                    o = dst[mt][:, c5 * 512:(c5 + 1) * 512]
                    for hb in (0, 64):
                        nc.vector.tensor_sub(
                            o[hb:hb + 32, :], ta[hb:hb + 32, :], ps[hb + 32:hb + 64, :])
                        nc.vector.tensor_add(
                            o[hb + 32:hb + 64, :], ps[hb:hb + 32, :], ta[hb + 32:hb + 64, :])

            # ---- v = x @ w_v in [s, d] layout --------------------------------
            def v_tiles(st0, st1, pool=None):
                for st in range(st0, st1):
                    ps = (pool or psacc).tile([128, 4, D], f32,
                                              tag="qp" if pool is None else "av",
                                              name="vps")
                    for k in range(KT):
                        nc.tensor.matmul(
                            ps[:, :, :],
                            lhsT=xt[:, k * S + st * 128: k * S + (st + 1) * 128],
                            rhs=wv[:, k * DHC:(k + 1) * DHC],
                            start=(k == 0), stop=(k == KT - 1),
                        )
                    # late tiles land in the ACT-hot window; DVE is idle there
                    cp = nc.scalar.copy if st < 8 else nc.vector.tensor_copy
                    cp(out=vsb[:, st * HPC:(st + 1) * HPC, 0:D],
                       in_=ps[:, :, :])

            # ---- causal attention per head, 1024-wide s-chunks ---------------
            # Two passes per (head, chunk): a dense scores+exp streak buffered
            # into SBUF et tiles, then a dense AV streak. Keeps the PE queue
            # free of mid-stream cross-engine waits so the HAM clock-gate can
            # reach 2.4GHz, with adjacent streams overlapping.
            def av_stream(h, jj, ets):
                # dense AV streak for the 512-wide stream (h, jj), then its
                # normalization chain: attn[d, s] = av[d, s] / av[64, s]
                mt, base = h // 2, (h % 2) * 64
                av = psav.tile([VW, 512], f32, tag="av", name="av")
                n_i = 4 * jj + 4
                for i in range(n_i):
                    et, c0 = ets[i]
                    lo = 512 * (jj % 2)
                    a = max(c0, lo) - lo
                    nc.tensor.matmul(
                        av[:, a:512],
                        lhsT=vsb[:, i * HPC + h, :],
                        rhs=et[:, lo + a:lo + 512],
                        start=(i == 0), stop=(i == n_i - 1),
                    )
                rc = bcp.tile([1, 512], f32, tag="rc")
                bc = bcp.tile([64, 512], f32, tag="bc")
                den = bcp.tile([1, 512], f32, tag="den")
                # custom DVE ops cannot read PSUM (silent garbage on HW):
                # stage the denominator row through SBUF first.
                if h < 2 or jj == 3:
                    nc.scalar.copy(out=den[:, :], in_=av[D:VW, :])
                else:
                    nc.vector.tensor_copy(out=den[:, :], in_=av[D:VW, :])
                nc.vector.reciprocal_approx_fast(rc[:, :], den[:, :])
                nc.gpsimd.partition_broadcast(bc[:, :], rc[:, :])
                nc.vector.tensor_mul(
                    attn[mt][base:base + 64, jj * 512:(jj + 1) * 512],
                    av[0:D, :], bc[:, :])

            def att_passA_range(h, j, i0, i1, ets):
                mt, base = h // 2, (h % 2) * 64
                q_t, k_t = qs[mt], ks[mt]
                if True:
                    n_i = 8 * j + 8
                    for i in range(i0, i1):
                        r = i - 8 * j
                        c0 = 128 * r if r >= 0 else 0
                        sp = psacc.tile([128, 1024], f32, tag="sp", name="sp")
                        for (a, b2) in ((c0, 512), (max(c0, 512), 1024)):
                            if a >= b2:
                                continue
                            nc.tensor.matmul(
                                sp[:, a:b2],
                                lhsT=k_t[base:base + 64, i * 128:(i + 1) * 128],
                                rhs=q_t[base:base + 64, j * 1024 + a: j * 1024 + b2],
                                start=True, stop=True,
                            )
                        et = expp.tile([128, 1024], bf16, tag="e")
                        nc.scalar.activation(
                            et[:, c0:1024], sp[:, c0:1024], Exp, scale=0.125)
                        if r >= 0:
                            nc.vector.tensor_mul(
                                et[:, c0:c0 + 128], et[:, c0:c0 + 128], mask[:, :])
                        ets.append((et, c0))

            def att_pair(h, j):
                # lower-half stream (cols [0:512) of this chunk-pair) only
                # needs i <= 8j+3: emit its AV streak mid-pass so it retires
                # while the upper half's scores are still streaming
                ets = []
                att_passA_range(h, j, 0, 8 * j + 4, ets)
                av_stream(h, 2 * j, ets)
                att_passA_range(h, j, 8 * j + 4, 8 * j + 8, ets)
                av_stream(h, 2 * j + 1, ets)

            # ---- partial out-projection: out = attn.T @ w_out ----------------
            def outproj(st0, st1, alt=False):
                for st in range(st0, st1):
                    for c2 in range(2):
                        if alt and (st % 2 == 1):
                            ps = psav.tile([128, 512], f32, tag="av", name="ops")
                        else:
                            ps = psacc.tile([128, 512], f32, tag="qp", name="ops")
                        for kt in range(2):
                            nc.tensor.matmul(
                                ps[:, :],
                                lhsT=attn[kt][:, st * 128:(st + 1) * 128],
                                rhs=wo[:, kt * E + c2 * 512: kt * E + (c2 + 1) * 512],
                                start=(kt == 0), stop=(kt == 1),
                            )
                        ot = outp.tile([128, 512], bf16, tag="o")
                        if st < 8 or st % 2 == 0:
                            nc.vector.tensor_copy(out=ot[:, :], in_=ps[:, :])
                        else:
                            nc.scalar.copy(out=ot[:, :], in_=ps[:, :])
                        nc.sync.dma_start(
                            out=out_e[st * 128:(st + 1) * 128, c2 * 512:(c2 + 1) * 512],
                            in_=ot[:, :])

            # ---- schedule: interleave phases so the PE queue stays dense -----
            # v tiles for s-cols [0:512) need only the first 1MB of xt:
            # they give the PE dense work during the DMA-bound head window
            v_tiles(0, 4, pool=psav)
            qk_tile(qs, wq, 0, 0)
            qk_tile(ks, wk, 0, 0)
            v_tiles(4, 8, pool=psav)
            # scores for heads 0/1 chunk 0 need only the first two roped
            # q/k column-pairs: they fill the PE while c5=2,3 rope runs
            att_pair(0, 0)
            att_pair(1, 0)
            # mt0 c5-2/3 ropes deferred: chunk-0 attention for heads 0/1 only
            # reads the first two column-pairs
            qk_tile(qs, wq, 0, 1)
            qk_tile(ks, wk, 0, 1)
            qk_tile(qs, wq, 1, 0)
            qk_tile(ks, wk, 1, 0)
            v_tiles(8, 16)
            # heads 0/1 chunk-1 only need mt0 + v: they fill the PE while the
            # mt1 rope occupies the vector engine. The mt1 c5-2/3 ropes are
            # deferred past them: chunk-0 attention for heads 2/3 only reads
            # the first two column-pairs.
            att_pair(0, 1)
            att_pair(1, 1)
            qk_tile(qs, wq, 1, 1)
            qk_tile(ks, wk, 1, 1)
            att_pair(2, 0)
            att_pair(3, 0)
            outproj(0, 8)
            e21 = []
            e31 = []
            att_passA_range(2, 1, 0, 12, e21)
            att_passA_range(3, 1, 0, 12, e31)
            av_stream(2, 2, e21)
            av_stream(3, 2, e31)
            att_passA_range(2, 1, 12, 16, e21)
            att_passA_range(3, 1, 12, 16, e31)
            outproj(8, 12)
            av_stream(2, 3, e21)
            av_stream(3, 3, e31)
            outproj(12, 16)

    nc.compile()
    return nc


def prep_inputs(x, w_qkv, w_out, freqs_cos, freqs_sin):
    """Shard + pre-tile the full fp32 inputs into 8 per-core in_maps."""
    cosT = np.ascontiguousarray(freqs_cos.T.astype(np.float32))  # [32, S]
    sinT = np.ascontiguousarray(freqs_sin.T.astype(np.float32))
    crep = np.tile(cosT, (4, 1))  # [128, S]
    srep = np.tile(sinT, (4, 1))
    mask = (np.arange(128)[:, None] <= np.arange(128)[None, :]).astype(BF16)

    xt_b = []
    for b in range(B):
        xt = np.ascontiguousarray(x[b].T)  # [E, S]
        xt_b.append(
            xt.reshape(KT, 128, S).transpose(1, 0, 2).reshape(128, KT * S)
            .astype(BF16))

    in_maps = []
    for c in range(N_CORES):
        b, hg = divmod(c, 4)
        cq, ck, cv = [], [], []
        for h in range(HPC):
            gh = hg * HPC + h
            base = gh * D
            perm = np.concatenate(
                [np.arange(base, base + D, 2), np.arange(base + 1, base + D, 2)])
            cq.append(perm)
            ck.append(perm + E)
            cv.append(np.arange(base, base + D) + 2 * E)

        def tile_w(cols):
            wc = w_qkv[:, np.concatenate(cols)]  # [E, 256]
            return (wc.reshape(KT, 128, DHC).transpose(1, 0, 2)
                    .reshape(128, KT * DHC).astype(BF16))

        wo_c = w_out[hg * DHC:(hg + 1) * DHC, :]  # [256, E]
        wo_p = (wo_c.reshape(2, 128, E).transpose(1, 0, 2)
                .reshape(128, 2 * E).astype(BF16))
        in_maps.append({
            "xt": xt_b[b],
            "wq": tile_w(cq),
            "wk": tile_w(ck),
            "wv": tile_w(cv),
            "wo": wo_p,
            "crep": crep,
            "srep": srep,
            "mask": mask,
        })
    return in_maps


_CACHE = {}


def _get_nc():
    if "nc" not in _CACHE:
        _inject_axon_hooks()
        _CACHE["nc"] = build()
    return _CACHE["nc"]


def kernel(x, w_qkv, w_out, freqs_cos, freqs_sin):
    from concourse.bass_utils import run_bass_kernel_spmd

    nc = _get_nc()
    in_maps = prep_inputs(
        np.asarray(x, dtype=np.float32),
        np.asarray(w_qkv, dtype=np.float32),
        np.asarray(w_out, dtype=np.float32),
        np.asarray(freqs_cos, dtype=np.float32),
        np.asarray(freqs_sin, dtype=np.float32),
    )
    res = run_bass_kernel_spmd(nc, in_maps, core_ids=list(range(N_CORES)))
    parts = [np.asarray(res.results[c]["out"], dtype=np.float32)
             for c in range(N_CORES)]
    out = np.stack([
        parts[0] + parts[1] + parts[2] + parts[3],
        parts[4] + parts[5] + parts[6] + parts[7],
    ]).astype(np.float32)
    return out


# revision 61
# speedup vs baseline: 1.0073x; 1.0073x over previous
"""Distributed causal-attention kernel for 8 TRN2 NeuronCores.

Reference computation (B=2, S=2048, E=1024, H=16, D=64):
  qkv = x @ w_qkv; split; rope(q), rope(k); causal softmax attention; out @ w_out

Sharding: data-parallel over batch (cores 0-3 -> batch 0, 4-7 -> batch 1),
tensor-parallel over heads (4 heads per core). Each core computes a partial
[S, E] out-projection; the host sums the 4 partials per batch.

All matmuls run in bf16 on the TensorEngine with fp32 PSUM accumulation.
Host-side prep pre-transposes x to [E, S], permutes q/k weight columns so the
RoPE pair interleave becomes two contiguous 32-row blocks per head, and
pre-tiles everything into [128, *] DMA-friendly layouts.
"""

import sys
import types

sys.path.insert(0, "/opt/trn_rl_repo")

import numpy as np
import ml_dtypes

BF16 = ml_dtypes.bfloat16

B, S, E, H, D = 2, 2048, 1024, 16, 64
N_CORES = 8
HPC = 4           # heads per core
DHC = HPC * D     # 256 head-dims per core
KT = E // 128     # 8 contraction tiles for the qkv projection
ST = S // 128     # 16 sequence tiles of 128
SC = S // 512     # 4 sequence chunks of 512
VW = D + 1        # 65: v plus the ones column (softmax denominator)


def _inject_axon_hooks():
    """Register the NTFF profile hook missing from this image's antenv so
    trace=True (or BASS_TRACE=1 in the env) doesn't crash run_bass_kernel_spmd."""
    if "antenv.axon_hooks" in sys.modules:
        return
    try:
        import antenv  # noqa: F401
    except Exception:
        return
    mod = types.ModuleType("antenv.axon_hooks")
    mod._hook = None

    def set_axon_ntff_profile_hook(h):
        mod._hook = h

    def get_axon_ntff_profile_hook():
        return mod._hook

    mod.set_axon_ntff_profile_hook = set_axon_ntff_profile_hook
    mod.get_axon_ntff_profile_hook = get_axon_ntff_profile_hook
    sys.modules["antenv.axon_hooks"] = mod
    try:
        from trn_agent_boot.trn_boot import _ntff_profile_via_ctypes

        mod._hook = _ntff_profile_via_ctypes("/opt/axon/libaxon_pjrt.so")
    except Exception:
        pass


def build():
    import concourse.bass as bass  # noqa: F401
    import concourse.mybir as mybir
    import concourse.tile as tile
    from concourse import bacc

    f32 = mybir.dt.float32
    bf16 = mybir.dt.bfloat16
    Exp = mybir.ActivationFunctionType.Exp

    nc = bacc.Bacc("TRN2", target_bir_lowering=False, debug=False,
                   num_devices=N_CORES)

    xt_e = nc.dram_tensor("xt", [128, KT * S], bf16, kind="ExternalInput").ap()
    wq_e = nc.dram_tensor("wq", [128, KT * DHC], bf16, kind="ExternalInput").ap()
    wk_e = nc.dram_tensor("wk", [128, KT * DHC], bf16, kind="ExternalInput").ap()
    wv_e = nc.dram_tensor("wv", [128, KT * DHC], bf16, kind="ExternalInput").ap()
    wo_e = nc.dram_tensor("wo", [128, 2 * E], bf16, kind="ExternalInput").ap()
    cr_e = nc.dram_tensor("crep", [128, S], f32, kind="ExternalInput").ap()
    sr_e = nc.dram_tensor("srep", [128, S], f32, kind="ExternalInput").ap()
    mk_e = nc.dram_tensor("mask", [128, 128], bf16, kind="ExternalInput").ap()
    out_e = nc.dram_tensor("out", [S, E], bf16, kind="ExternalOutput").ap()

    with tile.TileContext(nc) as tc:
        with (
            tc.tile_pool(name="static", bufs=1) as static,
            tc.tile_pool(name="tmp", bufs=2) as tmpp,
            tc.tile_pool(name="expp", bufs=32) as expp,
            tc.tile_pool(name="outp", bufs=4) as outp,
            tc.tile_pool(name="bcp", bufs=2) as bcp,
            tc.tile_pool(name="psacc", bufs=2, space="PSUM") as psacc,
            tc.tile_pool(name="psav", bufs=2, space="PSUM") as psav,
        ):
            xt = static.tile([128, KT * S], bf16, tag="xt")
            wq = static.tile([128, KT * DHC], bf16, tag="wq")
            wk = static.tile([128, KT * DHC], bf16, tag="wk")
            wv = static.tile([128, KT * DHC], bf16, tag="wv")
            wo = static.tile([128, 2 * E], bf16, tag="wo")
            crep = static.tile([128, S], f32, tag="crep")
            srep = static.tile([128, S], f32, tag="srep")
            mask = static.tile([128, 128], bf16, tag="mask")
            qs = [static.tile([128, S], bf16, tag=f"q{m}", name=f"q{m}") for m in range(2)]
            ks = [static.tile([128, S], bf16, tag=f"k{m}", name=f"k{m}") for m in range(2)]
            vsb = static.tile([128, ST * HPC, VW], bf16, tag="v")
            attn = [static.tile([128, S], bf16, tag=f"at{m}", name=f"at{m}") for m in range(2)]

            # Weights first (small, they gate the first matmuls), then xt
            # split into per-k-tile chunks so the k-accumulation can start
            # before the whole 4MB lands.
            nc.sync.dma_start(out=wq[:, :], in_=wq_e[:, :])
            nc.sync.dma_start(out=wk[:, :], in_=wk_e[:, :])
            nc.sync.dma_start(out=wv[:, :], in_=wv_e[:, :])
            nc.sync.dma_start(out=crep[:, :], in_=cr_e[:, :])
            nc.sync.dma_start(out=srep[:, :], in_=sr_e[:, :])
            nc.sync.dma_start(out=mask[:, :], in_=mk_e[:, :])
            # column-slice-major: all k-chunks of s-cols [512c5, 512c5+512)
            # land together, so the first qk tile (c5=0) completes ~14us
            # earlier than with whole-chunk streaming.
            for c5 in range(4):
                for k in range(KT):
                    nc.sync.dma_start(
                        out=xt[:, k * S + c5 * 512: k * S + (c5 + 1) * 512],
                        in_=xt_e[:, k * S + c5 * 512: k * S + (c5 + 1) * 512])
            nc.sync.dma_start(out=wo[:, :], in_=wo_e[:, :])

            nc.vector.memset(vsb[:, :, D:VW], 1.0)

            # ---- q, k projections + RoPE, [d, s] layout, 1024-wide chunks ----
            # m-tile rows: [hA_even(32) | hA_odd(32) | hB_even(32) | hB_odd(32)]
            def qk_tile(dst, w, mt, ch):
                # 512-wide halves on their own small psum tag so attention's
                # scores tiles never contend with rope-held qkv tiles.
                for hf in range(2):
                    c5 = 2 * ch + hf
                    ps = psacc.tile([128, 512], f32, tag="qp", name="qp")
                    for k in range(KT):
                        nc.tensor.matmul(
                            ps[:, :],
                            lhsT=w[:, k * DHC + mt * 128: k * DHC + (mt + 1) * 128],
                            rhs=xt[:, k * S + c5 * 512: k * S + (c5 + 1) * 512],
                            start=(k == 0), stop=(k == KT - 1),
                        )
                    # ta (SBUF) = q*cos, then ps (PSUM) *= sin in place;
                    # the rotate combines then always mix one SBUF and one
                    # PSUM operand, which the BIR verifier allows at
                    # different base partitions (SB+SB it does not).
                    ta = tmpp.tile([128, 512], f32, tag="ta")
                    c_s = crep[:, c5 * 512:(c5 + 1) * 512]
                    s_s = srep[:, c5 * 512:(c5 + 1) * 512]
                    nc.vector.tensor_mul(ta[:, :], ps[:, :], c_s)
                    nc.vector.tensor_mul(ps[:, :], ps[:, :], s_s)
                    o = dst[mt][:, c5 * 512:(c5 + 1) * 512]
                    for hb in (0, 64):
                        nc.vector.tensor_sub(
                            o[hb:hb + 32, :], ta[hb:hb + 32, :], ps[hb + 32:hb + 64, :])
                        nc.vector.tensor_add(
                            o[hb + 32:hb + 64, :], ps[hb:hb + 32, :], ta[hb + 32:hb + 64, :])

            # ---- v = x @ w_v in [s, d] layout --------------------------------
            def v_tiles(st0, st1, pool=None):
                for st in range(st0, st1):
                    ps = (pool or psacc).tile([128, 4, D], f32,
                                              tag="qp" if pool is None else "av",
                                              name="vps")
                    for k in range(KT):
                        nc.tensor.matmul(
                            ps[:, :, :],
                            lhsT=xt[:, k * S + st * 128: k * S + (st + 1) * 128],
                            rhs=wv[:, k * DHC:(k + 1) * DHC],
                            start=(k == 0), stop=(k == KT - 1),
                        )
                    nc.scalar.copy(out=vsb[:, st * HPC:(st + 1) * HPC, 0:D],
                                   in_=ps[:, :, :])

            # ---- causal attention per head, 1024-wide s-chunks ---------------
            # Two passes per (head, chunk): a dense scores+exp streak buffered
            # into SBUF et tiles, then a dense AV streak. Keeps the PE queue
            # free of mid-stream cross-engine waits so the HAM clock-gate can
            # reach 2.4GHz, with adjacent streams overlapping.
            def av_stream(h, jj, ets):
                # dense AV streak for the 512-wide stream (h, jj), then its
                # normalization chain: attn[d, s] = av[d, s] / av[64, s]
                mt, base = h // 2, (h % 2) * 64
                av = psav.tile([VW, 512], f32, tag="av", name="av")
                n_i = 4 * jj + 4
                for i in range(n_i):
                    et, c0 = ets[i]
                    lo = 512 * (jj % 2)
                    a = max(c0, lo) - lo
                    nc.tensor.matmul(
                        av[:, a:512],
                        lhsT=vsb[:, i * HPC + h, :],
                        rhs=et[:, lo + a:lo + 512],
                        start=(i == 0), stop=(i == n_i - 1),
                    )
                rc = bcp.tile([1, 512], f32, tag="rc")
                bc = bcp.tile([64, 512], f32, tag="bc")
                den = bcp.tile([1, 512], f32, tag="den")
                # custom DVE ops cannot read PSUM (silent garbage on HW):
                # stage the denominator row through SBUF first.
                if h < 2 or jj == 3:
                    nc.scalar.copy(out=den[:, :], in_=av[D:VW, :])
                else:
                    nc.vector.tensor_copy(out=den[:, :], in_=av[D:VW, :])
                nc.vector.reciprocal_approx_fast(rc[:, :], den[:, :])
                nc.gpsimd.partition_broadcast(bc[:, :], rc[:, :])
                nc.vector.tensor_mul(
                    attn[mt][base:base + 64, jj * 512:(jj + 1) * 512],
                    av[0:D, :], bc[:, :])

            def att_passA_range(h, j, i0, i1, ets):
                mt, base = h // 2, (h % 2) * 64
                q_t, k_t = qs[mt], ks[mt]
                if True:
                    n_i = 8 * j + 8
                    for i in range(i0, i1):
                        r = i - 8 * j
                        c0 = 128 * r if r >= 0 else 0
                        sp = psacc.tile([128, 1024], f32, tag="sp", name="sp")
                        for (a, b2) in ((c0, 512), (max(c0, 512), 1024)):
                            if a >= b2:
                                continue
                            nc.tensor.matmul(
                                sp[:, a:b2],
                                lhsT=k_t[base:base + 64, i * 128:(i + 1) * 128],
                                rhs=q_t[base:base + 64, j * 1024 + a: j * 1024 + b2],
                                start=True, stop=True,
                            )
                        et = expp.tile([128, 1024], bf16, tag="e")
                        nc.scalar.activation(
                            et[:, c0:1024], sp[:, c0:1024], Exp, scale=0.125)
                        if r >= 0:
                            nc.vector.tensor_mul(
                                et[:, c0:c0 + 128], et[:, c0:c0 + 128], mask[:, :])
                        ets.append((et, c0))

            def att_pair(h, j):
                # lower-half stream (cols [0:512) of this chunk-pair) only
                # needs i <= 8j+3: emit its AV streak mid-pass so it retires
                # while the upper half's scores are still streaming
                ets = []
                att_passA_range(h, j, 0, 8 * j + 4, ets)
                av_stream(h, 2 * j, ets)
                att_passA_range(h, j, 8 * j + 4, 8 * j + 8, ets)
                av_stream(h, 2 * j + 1, ets)

            # ---- partial out-projection: out = attn.T @ w_out ----------------
            def outproj(st0, st1, alt=False):
                for st in range(st0, st1):
                    for c2 in range(2):
                        if alt and (st % 2 == 1):
                            ps = psav.tile([128, 512], f32, tag="av", name="ops")
                        else:
                            ps = psacc.tile([128, 512], f32, tag="qp", name="ops")
                        for kt in range(2):
                            nc.tensor.matmul(
                                ps[:, :],
                                lhsT=attn[kt][:, st * 128:(st + 1) * 128],
                                rhs=wo[:, kt * E + c2 * 512: kt * E + (c2 + 1) * 512],
                                start=(kt == 0), stop=(kt == 1),
                            )
                        ot = outp.tile([128, 512], bf16, tag="o")
                        if st < 8 or st % 2 == 0:
                            nc.vector.tensor_copy(out=ot[:, :], in_=ps[:, :])
                        else:
                            nc.scalar.copy(out=ot[:, :], in_=ps[:, :])
                        nc.sync.dma_start(
                            out=out_e[st * 128:(st + 1) * 128, c2 * 512:(c2 + 1) * 512],
                            in_=ot[:, :])

            # ---- schedule: interleave phases so the PE queue stays dense -----
            # v tiles for s-cols [0:512) need only the first 1MB of xt:
            # they give the PE dense work during the DMA-bound head window
            v_tiles(0, 4, pool=psav)
            qk_tile(qs, wq, 0, 0)
            qk_tile(ks, wk, 0, 0)
            v_tiles(4, 8, pool=psav)
            # scores for heads 0/1 chunk 0 need only the first two roped
            # q/k column-pairs: they fill the PE while c5=2,3 rope runs
            att_pair(0, 0)
            att_pair(1, 0)
            # mt0 c5-2/3 ropes deferred: chunk-0 attention for heads 0/1 only
            # reads the first two column-pairs
            qk_tile(qs, wq, 0, 1)
            qk_tile(ks, wk, 0, 1)
            qk_tile(qs, wq, 1, 0)
            qk_tile(ks, wk, 1, 0)
            v_tiles(8, 16)
            # heads 0/1 chunk-1 only need mt0 + v: they fill the PE while the
            # mt1 rope occupies the vector engine. The mt1 c5-2/3 ropes are
            # deferred past them: chunk-0 attention for heads 2/3 only reads
            # the first two column-pairs.
            att_pair(0, 1)
            att_pair(1, 1)
            qk_tile(qs, wq, 1, 1)
            qk_tile(ks, wk, 1, 1)
            att_pair(2, 0)
            att_pair(3, 0)
            outproj(0, 8)
            e21 = []
            e31 = []
            att_passA_range(2, 1, 0, 12, e21)
            att_passA_range(3, 1, 0, 12, e31)
            av_stream(2, 2, e21)
            av_stream(3, 2, e31)
            att_passA_range(2, 1, 12, 16, e21)
            att_passA_range(3, 1, 12, 16, e31)
            outproj(8, 12)
            av_stream(2, 3, e21)
            av_stream(3, 3, e31)
            outproj(12, 16)

    nc.compile()
    return nc


def prep_inputs(x, w_qkv, w_out, freqs_cos, freqs_sin):
    """Shard + pre-tile the full fp32 inputs into 8 per-core in_maps."""
    cosT = np.ascontiguousarray(freqs_cos.T.astype(np.float32))  # [32, S]
    sinT = np.ascontiguousarray(freqs_sin.T.astype(np.float32))
    crep = np.tile(cosT, (4, 1))  # [128, S]
    srep = np.tile(sinT, (4, 1))
    mask = (np.arange(128)[:, None] <= np.arange(128)[None, :]).astype(BF16)

    xt_b = []
    for b in range(B):
        xt = np.ascontiguousarray(x[b].T)  # [E, S]
        xt_b.append(
            xt.reshape(KT, 128, S).transpose(1, 0, 2).reshape(128, KT * S)
            .astype(BF16))

    in_maps = []
    for c in range(N_CORES):
        b, hg = divmod(c, 4)
        cq, ck, cv = [], [], []
        for h in range(HPC):
            gh = hg * HPC + h
            base = gh * D
            perm = np.concatenate(
                [np.arange(base, base + D, 2), np.arange(base + 1, base + D, 2)])
            cq.append(perm)
            ck.append(perm + E)
            cv.append(np.arange(base, base + D) + 2 * E)

        def tile_w(cols):
            wc = w_qkv[:, np.concatenate(cols)]  # [E, 256]
            return (wc.reshape(KT, 128, DHC).transpose(1, 0, 2)
                    .reshape(128, KT * DHC).astype(BF16))

        wo_c = w_out[hg * DHC:(hg + 1) * DHC, :]  # [256, E]
        wo_p = (wo_c.reshape(2, 128, E).transpose(1, 0, 2)
                .reshape(128, 2 * E).astype(BF16))
        in_maps.append({
            "xt": xt_b[b],
            "wq": tile_w(cq),
            "wk": tile_w(ck),
            "wv": tile_w(cv),
            "wo": wo_p,
            "crep": crep,
            "srep": srep,
            "mask": mask,
        })
    return in_maps


_CACHE = {}


def _get_nc():
    if "nc" not in _CACHE:
        _inject_axon_hooks()
        _CACHE["nc"] = build()
    return _CACHE["nc"]


def kernel(x, w_qkv, w_out, freqs_cos, freqs_sin):
    from concourse.bass_utils import run_bass_kernel_spmd

    nc = _get_nc()
    in_maps = prep_inputs(
        np.asarray(x, dtype=np.float32),
        np.asarray(w_qkv, dtype=np.float32),
        np.asarray(w_out, dtype=np.float32),
        np.asarray(freqs_cos, dtype=np.float32),
        np.asarray(freqs_sin, dtype=np.float32),
    )
    res = run_bass_kernel_spmd(nc, in_maps, core_ids=list(range(N_CORES)))
    parts = [np.asarray(res.results[c]["out"], dtype=np.float32)
             for c in range(N_CORES)]
    out = np.stack([
        parts[0] + parts[1] + parts[2] + parts[3],
        parts[4] + parts[5] + parts[6] + parts[7],
    ]).astype(np.float32)
    return out
